# revision 4
# baseline (speedup 1.0000x reference)
"""Complex dot-product attention on 8 Trainium2 NeuronCores.

Reference computation (per batch b):
    sr = (qr @ kr^T - qi @ ki^T) / sqrt(D)      si = (qr @ ki^T + qi @ kr^T) / sqrt(D)
    ar = softmax(sr, axis=k)                    ai = softmax(si, axis=k)
    out_r = ar @ vr - ai @ vi                   out_i = ar @ vi + ai @ vr

Shapes: q/k/v [B=4, S=4096, D=64, 2] fp32, interleaved (real, imag) last dim.

Sharding: data-parallel over batch x sequence-parallel over query rows.
Core c handles batch b = c//2, query rows [h*2048, (h+1)*2048) with h = c%2,
and all 4096 keys of that batch (K/V replicated per batch pair). No
collectives; the host slices inputs per core and concatenates outputs.

Per-core kernel math trick: with Q, K, V kept in their NATURAL interleaved
layout ([s, 2d] where col 2d = real_d, col 2d+1 = imag_d):
    sr[q,k] = sum_{2d} Qneg[q,:] * K[k,:]   with Qneg = [qr0, -qi0, qr1, -qi1, ...]
    si[q,k] = sum_{2d} Qswap[q,:] * K[k,:]  with Qswap = [qi0, qr0, qi1, qr1, ...]
so both score components contract over the full 128-wide interleaved axis
against the SAME natural K. Scores are computed TRANSPOSED ([k, q]) so that
the attention matmul (contraction over k) can consume the exp'd scores
directly from SBUF as the moving operand:
    P_a[m, q] = sum_k V[k, m]  * Er[k, q]   (V natural as stationary)
    P_b[m, q] = sum_k V2[k, m] * Ei[k, q]   (V2 = [-vi0, vr0, -vi1, vr1, ...])
    out_T[m, q] = P_a[m,q] / sum_r[q] + P_b[m,q] / sum_i[q]
which lands rows m = (d, complex)-interleaved, exactly the HBM layout after a
final 128x128 PE transpose. Softmax skips max-subtraction (scores are
O(+-6) for randn inputs; exp stays comfortably inside fp32 range).
"""

import os

import numpy as np

import concourse.bass as bass
import concourse.mybir as mybir
import concourse.tile as tile
from concourse import bacc

F32 = mybir.dt.float32
F32R = mybir.dt.float32r
EXP = mybir.ActivationFunctionType.Exp
MULT = mybir.AluOpType.mult
ADD = mybir.AluOpType.add

B, S, D = 4, 4096, 64
W = 2 * D  # 128 interleaved columns
NCORES = 8
SQ = B * S // NCORES  # 2048 query rows per core
SCALE = 1.0 / float(np.sqrt(D))


def build_nc(sq=SQ, sk=S, gk=2, qb_size=512):
    """Build the per-core SPMD bass program."""
    nq = sq // 128   # q 128-row chunks
    nk = sk // 128   # k tiles
    nqb = sq // qb_size
    njb = qb_size // 128
    ngroups = nk // gk

    nc = bacc.Bacc(target_bir_lowering=False)

    q_d = nc.declare_dram_parameter("q", [sq, W], F32, isOutput=False)
    k_d = nc.declare_dram_parameter("k", [sk, W], F32, isOutput=False)
    v_d = nc.declare_dram_parameter("v", [sk, W], F32, isOutput=False)
    ident_d = nc.declare_dram_parameter("ident", [128, 128], F32, isOutput=False)
    swapneg_d = nc.declare_dram_parameter("swapneg", [128, 128], F32R, isOutput=False)
    onesm_d = nc.declare_dram_parameter("onesm", [128, 128], F32R, isOutput=False)
    sign_d = nc.declare_dram_parameter("sign", [128, 1], F32, isOutput=False)
    out_d = nc.declare_dram_parameter("out", [sq, W], F32, isOutput=True)

    qv = q_d.rearrange("(c p) n -> p c n", p=128)  # [128, nq, 128]
    kv = k_d.rearrange("(c p) n -> p c n", p=128)
    vv = v_d.rearrange("(c p) n -> p c n", p=128)
    # out row = a*qb_size + j*128 + p
    ov = out_d.rearrange("(a j p) n -> a p j n", p=128, j=njb)

    with tile.TileContext(nc) as tc:
        with (
            tc.tile_pool(name="const", bufs=1) as constp,
            tc.tile_pool(name="big", bufs=1) as big,
            tc.tile_pool(name="epool", bufs=3) as epool,
            tc.tile_pool(name="small", bufs=2) as small,
            # PSUM budget: 8 banks of [128 x 512 fp32].
            tc.tile_pool(name="psA", bufs=2, space=bass.MemorySpace.PSUM) as psA,  # scores: 2x2 banks
            tc.tile_pool(name="psB", bufs=2, space=bass.MemorySpace.PSUM) as psB,  # AV accum: 2x1
            tc.tile_pool(name="psC", bufs=2, space=bass.MemorySpace.PSUM) as psC,  # sums: 2x1
        ):
            ident = constp.tile([128, 128], F32, tag="ident")
            nc.sync.dma_start(ident[:], ident_d[:])
            swapneg = constp.tile([128, 128], F32R, tag="swapneg")
            nc.sync.dma_start(swapneg[:], swapneg_d[:])
            onesm = constp.tile([128, 128], F32R, tag="onesm")
            nc.sync.dma_start(onesm[:], onesm_d[:])
            sign = constp.tile([128, 1], F32, tag="sign")
            nc.sync.dma_start(sign[:], sign_d[:])

            qnat = big.tile([128, nq, 128], F32, tag="qnat")
            for c0 in range(0, nq, 8):
                ce = min(c0 + 8, nq)
                nc.sync.dma_start(qnat[:, c0:ce, :], qv[:, c0:ce, :])
            knat = big.tile([128, nk, 128], F32, tag="knat")
            for c0 in range(0, nk, 8):
                ce = min(c0 + 8, nk)
                nc.sync.dma_start(knat[:, c0:ce, :], kv[:, c0:ce, :])
            vnat = big.tile([128, nk, 128], F32, tag="vnat")
            for c0 in range(0, nk, 8):
                ce = min(c0 + 8, nk)
                nc.sync.dma_start(vnat[:, c0:ce, :], vv[:, c0:ce, :])

            # K^T: [2d, k]
            kT = big.tile([128, sk], F32R, tag="kT")
            for c in range(nk):
                ps = psA.tile([128, 128], F32, tag="sc")
                nc.tensor.transpose(ps[:], knat[:, c, :], ident[:])
                nc.vector.tensor_copy(kT[:, c * 128:(c + 1) * 128], ps[:])

            # Qneg^T: [2d, q] with odd partitions negated (sign = +1/-1 per partition)
            qnegT = big.tile([128, sq], F32R, tag="qnegT")
            for c in range(nq):
                ps = psA.tile([128, 128], F32, tag="sc")
                nc.tensor.transpose(ps[:], qnat[:, c, :], ident[:])
                nc.vector.tensor_scalar(
                    out=qnegT[:, c * 128:(c + 1) * 128], in0=ps[:],
                    scalar1=sign[:], scalar2=None, op0=MULT,
                )

            # Qswap^T = M' @ Qneg^T (M' undoes the sign and swaps even/odd partitions)
            qswapT = big.tile([128, sq], F32R, tag="qswapT")
            for n0 in range(0, sq, 512):
                ps = psB.tile([128, 512], F32, tag="pav")
                nc.tensor.matmul(
                    ps[:], swapneg[:],
                    qnegT[:, n0:n0 + 512],
                )
                nc.vector.tensor_copy(qswapT[:, n0:n0 + 512], ps[:])

            # V rounded to fp32r for the AV-matmul stationary
            v1r = big.tile([128, nk, 128], F32R, tag="v1r")
            nc.vector.tensor_copy(v1r[:], vnat[:])

            # V2 = [-vi, vr] interleaved
            v2 = big.tile([128, nk, 128], F32R, tag="v2")
            vp = vnat.rearrange("p c (d two) -> p c d two", two=2)
            v2p = v2.rearrange("p c (d two) -> p c d two", two=2)
            nc.vector.tensor_scalar(
                out=v2p[:, :, :, 0], in0=vp[:, :, :, 1],
                scalar1=-1.0, scalar2=None, op0=MULT,
            )
            nc.vector.tensor_copy(v2p[:, :, :, 1], vp[:, :, :, 0])

            def pe_consume(prev, pav, psum, vsrc):
                """AV + denominator matmuls for one exp'd score group."""
                et, g = prev
                for j in range(gk):
                    kt = g * gk + j
                    er = et[:, j * 512:(j + 1) * 512]
                    nc.tensor.matmul(
                        pav[:], vsrc[:, kt, :], er,
                        start=(kt == 0), stop=(kt == nk - 1),
                    )
                    nc.tensor.matmul(
                        psum[:], onesm[:], er,
                        start=(kt == 0), stop=(kt == nk - 1),
                    )

            for qb in range(nqb):
                pavs, rhos = [], []
                for comp in range(2):
                    src = qnegT if comp == 0 else qswapT
                    rhs_q = src[:, qb * qb_size:(qb + 1) * qb_size]
                    vsrc = v1r if comp == 0 else v2
                    pav = psB.tile([128, qb_size], F32, tag="pav")
                    psum = psC.tile([128, qb_size], F32, tag="sum")
                    prev = None
                    for g in range(ngroups):
                        sc = psA.tile([128, gk * 512], F32, tag="sc")
                        for j in range(gk):
                            kt = g * gk + j
                            nc.tensor.matmul(
                                sc[:, j * 512:(j + 1) * 512],
                                kT[:, kt * 128:(kt + 1) * 128],
                                rhs_q,
                            )
                        # drain previous group's exp through AV/sum matmuls so
                        # PE's scores for group g+1 overlap ACT's exp of group g
                        if prev is not None:
                            pe_consume(prev, pav, psum, vsrc)
                        et = epool.tile([128, gk * 512], F32R, tag="e")
                        nc.scalar.activation(et[:], sc[:], EXP, scale=SCALE)
                        prev = (et, g)
                    pe_consume(prev, pav, psum, vsrc)
                    rho = small.tile([128, qb_size], F32, tag=f"rho{comp}")
                    nc.vector.reciprocal(rho[:], psum[:])
                    pavs.append(pav)
                    rhos.append(rho)

                t0 = small.tile([128, qb_size], F32, tag="t0")
                nc.vector.tensor_tensor(out=t0[:], in0=pavs[0][:], in1=rhos[0][:], op=MULT)
                t1 = small.tile([128, qb_size], F32, tag="t1")
                nc.vector.tensor_tensor(out=t1[:], in0=pavs[1][:], in1=rhos[1][:], op=MULT)
                o = small.tile([128, qb_size], F32, tag="o")
                nc.vector.tensor_tensor(out=o[:], in0=t0[:], in1=t1[:], op=ADD)

                osb = small.tile([128, njb, 128], F32, tag="osb")
                pt = psA.tile([128, gk * 512], F32, tag="sc")
                for j in range(njb):
                    nc.tensor.transpose(
                        pt[:, j * 128:(j + 1) * 128], o[:, j * 128:(j + 1) * 128],
                        ident[:],
                    )
                    nc.vector.tensor_copy(osb[:, j, :], pt[:, j * 128:(j + 1) * 128])
                nc.sync.dma_start(ov[qb], osb[:])

    nc.compile()
    return nc


def host_consts():
    ident = np.eye(128, dtype=np.float32)
    # lhsT for Qswap^T = M' @ Qneg^T: lhsT[2d+1, 2d] = -1, lhsT[2d, 2d+1] = +1
    swapneg = np.zeros((128, 128), dtype=np.float32)
    idx = np.arange(0, 128, 2)
    swapneg[idx + 1, idx] = -1.0
    swapneg[idx, idx + 1] = 1.0
    onesm = np.ones((128, 128), dtype=np.float32)
    sign = np.empty((128, 1), dtype=np.float32)
    sign[0::2] = 1.0
    sign[1::2] = -1.0
    return {"ident": ident, "swapneg": swapneg, "onesm": onesm, "sign": sign}


_LAST_RESULTS = [None]  # BassKernelResults stash for test harness introspection


def kernel(queries, keys, values):
    from concourse.bass_utils import run_bass_kernel_spmd

    queries = np.ascontiguousarray(np.asarray(queries, dtype=np.float32))
    keys = np.ascontiguousarray(np.asarray(keys, dtype=np.float32))
    values = np.ascontiguousarray(np.asarray(values, dtype=np.float32))
    assert queries.shape == (B, S, D, 2), queries.shape

    nc = build_nc()
    consts = host_consts()
    halves = S // (NCORES // B)  # 2048 rows per core
    in_maps = []
    for c in range(NCORES):
        b, h = c // 2, c % 2
        in_maps.append({
            "q": queries[b, h * halves:(h + 1) * halves].reshape(SQ, W),
            "k": keys[b].reshape(S, W),
            "v": values[b].reshape(S, W),
            **consts,
        })
    res = run_bass_kernel_spmd(
        nc, in_maps, list(range(NCORES)),
        trace=bool(int(os.environ.get("KERNEL_TRACE", "0"))),
    )
    _LAST_RESULTS[0] = res
    out = np.empty((B, S, D, 2), dtype=np.float32)
    for c in range(NCORES):
        b, h = c // 2, c % 2
        out[b, h * halves:(h + 1) * halves] = res.results[c]["out"].reshape(halves, D, 2)
    return out


# revision 6
# speedup vs baseline: 1.0555x; 1.0555x over previous
"""Complex dot-product attention on 8 Trainium2 NeuronCores.

Reference computation (per batch b):
    sr = (qr @ kr^T - qi @ ki^T) / sqrt(D)      si = (qr @ ki^T + qi @ kr^T) / sqrt(D)
    ar = softmax(sr, axis=k)                    ai = softmax(si, axis=k)
    out_r = ar @ vr - ai @ vi                   out_i = ar @ vi + ai @ vr

Shapes: q/k/v [B=4, S=4096, D=64, 2] fp32, interleaved (real, imag) last dim.

Sharding: data-parallel over batch x sequence-parallel over query rows.
Core c handles batch b = c//2, query rows [h*2048, (h+1)*2048) with h = c%2,
and all 4096 keys of that batch (K/V replicated per batch pair). No
collectives; the host slices inputs per core and concatenates outputs.

Per-core kernel math trick: with Q, K, V kept in their NATURAL interleaved
layout ([s, 2d] where col 2d = real_d, col 2d+1 = imag_d):
    sr[q,k] = sum_{2d} Qneg[q,:] * K[k,:]   with Qneg = [qr0, -qi0, qr1, -qi1, ...]
    si[q,k] = sum_{2d} Qswap[q,:] * K[k,:]  with Qswap = [qi0, qr0, qi1, qr1, ...]
so both score components contract over the full 128-wide interleaved axis
against the SAME natural K. Scores are computed TRANSPOSED ([k, q]) so that
the attention matmul (contraction over k) can consume the exp'd scores
directly from SBUF as the moving operand:
    P_a[m, q] = sum_k V[k, m]  * Er[k, q]   (V natural as stationary)
    P_b[m, q] = sum_k V2[k, m] * Ei[k, q]   (V2 = [-vi0, vr0, -vi1, vr1, ...])
    out_T[m, q] = P_a[m,q] / sum_r[q] + P_b[m,q] / sum_i[q]
which lands rows m = (d, complex)-interleaved, exactly the HBM layout after a
final 128x128 PE transpose. Softmax skips max-subtraction (scores are
O(+-6) for randn inputs; exp stays comfortably inside fp32 range).
"""

import os

import numpy as np

import concourse.bass as bass
import concourse.mybir as mybir
import concourse.tile as tile
from concourse import bacc

F32 = mybir.dt.float32
F32R = mybir.dt.float32r
EXP = mybir.ActivationFunctionType.Exp
MULT = mybir.AluOpType.mult
ADD = mybir.AluOpType.add

B, S, D = 4, 4096, 64
W = 2 * D  # 128 interleaved columns
NCORES = 8
SQ = B * S // NCORES  # 2048 query rows per core
SCALE = 1.0 / float(np.sqrt(D))


def build_nc(sq=SQ, sk=S, gk=2, qb_size=512):
    """Build the per-core SPMD bass program."""
    nq = sq // 128   # q 128-row chunks
    nk = sk // 128   # k tiles
    nqb = sq // qb_size
    njb = qb_size // 128
    ngroups = nk // gk

    nc = bacc.Bacc(target_bir_lowering=False)

    q_d = nc.declare_dram_parameter("q", [sq, W], F32, isOutput=False)
    k_d = nc.declare_dram_parameter("k", [sk, W], F32, isOutput=False)
    v_d = nc.declare_dram_parameter("v", [sk, W], F32, isOutput=False)
    ident_d = nc.declare_dram_parameter("ident", [128, 128], F32, isOutput=False)
    swapneg_d = nc.declare_dram_parameter("swapneg", [128, 128], F32R, isOutput=False)
    onesm_d = nc.declare_dram_parameter("onesm", [128, 128], F32R, isOutput=False)
    sign_d = nc.declare_dram_parameter("sign", [128, 1], F32, isOutput=False)
    out_d = nc.declare_dram_parameter("out", [sq, W], F32, isOutput=True)

    qv = q_d.rearrange("(c p) n -> p c n", p=128)  # [128, nq, 128]
    kv = k_d.rearrange("(c p) n -> p c n", p=128)
    vv = v_d.rearrange("(c p) n -> p c n", p=128)
    # out row = a*qb_size + j*128 + p
    ov = out_d.rearrange("(a j p) n -> a p j n", p=128, j=njb)

    with tile.TileContext(nc) as tc:
        with (
            tc.tile_pool(name="const", bufs=1) as constp,
            tc.tile_pool(name="big", bufs=1) as big,
            tc.tile_pool(name="epool", bufs=4) as epool,
            tc.tile_pool(name="small", bufs=2) as small,
            # PSUM budget: 8 banks of [128 x 512 fp32].
            tc.tile_pool(name="psA", bufs=2, space=bass.MemorySpace.PSUM) as psA,  # scores: 2x2 banks
            tc.tile_pool(name="psB", bufs=2, space=bass.MemorySpace.PSUM) as psB,  # AV accum: 2x1
            tc.tile_pool(name="psC", bufs=2, space=bass.MemorySpace.PSUM) as psC,  # sums: 2x1
        ):
            ident = constp.tile([128, 128], F32, tag="ident")
            nc.sync.dma_start(ident[:], ident_d[:])
            swapneg = constp.tile([128, 128], F32R, tag="swapneg")
            nc.sync.dma_start(swapneg[:], swapneg_d[:])
            onesm = constp.tile([128, 128], F32R, tag="onesm")
            nc.sync.dma_start(onesm[:], onesm_d[:])
            sign = constp.tile([128, 1], F32, tag="sign")
            nc.sync.dma_start(sign[:], sign_d[:])

            CH = 8  # tiles per DMA chunk
            kchunks, qchunks, vchunks = [], [], []
            for c0 in range(0, nk, CH):
                t = big.tile([128, min(CH, nk - c0), 128], F32, tag=f"knat{c0}")
                nc.sync.dma_start(t[:], kv[:, c0:c0 + t.shape[1], :])
                kchunks.append(t)
            for c0 in range(0, nq, CH):
                t = big.tile([128, min(CH, nq - c0), 128], F32, tag=f"qnat{c0}")
                nc.sync.dma_start(t[:], qv[:, c0:c0 + t.shape[1], :])
                qchunks.append(t)

            # K^T: [2d, k]
            kT = big.tile([128, sk], F32R, tag="kT")
            for c in range(nk):
                ps = psA.tile([128, 128], F32, tag="sc")
                nc.tensor.transpose(ps[:], kchunks[c // CH][:, c % CH, :], ident[:])
                nc.vector.tensor_copy(kT[:, c * 128:(c + 1) * 128], ps[:])

            # Qneg^T: [2d, q] with odd partitions negated (sign = +1/-1 per partition)
            qnegT = big.tile([128, sq], F32R, tag="qnegT")
            for c in range(nq):
                ps = psA.tile([128, 128], F32, tag="sc")
                nc.tensor.transpose(ps[:], qchunks[c // CH][:, c % CH, :], ident[:])
                nc.vector.tensor_scalar(
                    out=qnegT[:, c * 128:(c + 1) * 128], in0=ps[:],
                    scalar1=sign[:], scalar2=None, op0=MULT,
                )

            vnat = big.tile([128, nk, 128], F32, tag="vnat")
            for c0 in range(0, nk, CH):
                ce = min(c0 + CH, nk)
                nc.sync.dma_start(vnat[:, c0:ce, :], vv[:, c0:ce, :])

            # Qswap^T = M' @ Qneg^T (M' undoes the sign and swaps even/odd partitions)
            qswapT = big.tile([128, sq], F32R, tag="qswapT")
            for n0 in range(0, sq, 512):
                ps = psB.tile([128, 512], F32, tag="pav")
                nc.tensor.matmul(
                    ps[:], swapneg[:],
                    qnegT[:, n0:n0 + 512],
                )
                nc.vector.tensor_copy(qswapT[:, n0:n0 + 512], ps[:])

            # V rounded to fp32r for the AV-matmul stationary
            v1r = big.tile([128, nk, 128], F32R, tag="v1r")
            nc.vector.tensor_copy(v1r[:], vnat[:])

            # V2 = [-vi, vr] interleaved
            v2 = big.tile([128, nk, 128], F32R, tag="v2")
            vp = vnat.rearrange("p c (d two) -> p c d two", two=2)
            v2p = v2.rearrange("p c (d two) -> p c d two", two=2)
            nc.vector.tensor_scalar(
                out=v2p[:, :, :, 0], in0=vp[:, :, :, 1],
                scalar1=-1.0, scalar2=None, op0=MULT,
            )
            nc.vector.tensor_copy(v2p[:, :, :, 1], vp[:, :, :, 0])

            def pe_consume(prev, pav, psum, vsrc, pairs):
                """AV matmuls + pair-reduction for one exp'd score group.

                Denominator: E slices are pair-added (DVE/GpSimd, alternating),
                pairs quad-merged, and only one 128x512 ones-matmul per quad
                streams through the PE -- 4x less PE time than per-kt ones.
                """
                et, g = prev
                for j in range(gk):
                    kt = g * gk + j
                    er = et[:, j * 512:(j + 1) * 512]
                    nc.tensor.matmul(
                        pav[:], vsrc[:, kt, :], er,
                        start=(kt == 0), stop=(kt == nk - 1),
                    )
                pr = small.tile([128, qb_size], F32R, tag=f"pair{g % 3}")
                eng = nc.vector if g % 2 == 0 else nc.gpsimd
                eng.tensor_tensor(out=pr[:], in0=et[:, 0:512], in1=et[:, 512:1024], op=ADD)
                pairs.append(pr)
                if len(pairs) == 2:
                    qd = small.tile([128, qb_size], F32R, tag=f"quad{(g // 2) % 2}")
                    nc.vector.tensor_tensor(out=qd[:], in0=pairs[0][:], in1=pairs[1][:], op=ADD)
                    pairs.clear()
                    h = g // 2
                    nc.tensor.matmul(
                        psum[:], onesm[:], qd[:],
                        start=(h == 0), stop=(h == ngroups // 2 - 1),
                    )

            for qb in range(nqb):
                pavs, rhos = [], []
                for comp in range(2):
                    src = qnegT if comp == 0 else qswapT
                    rhs_q = src[:, qb * qb_size:(qb + 1) * qb_size]
                    vsrc = v1r if comp == 0 else v2
                    pav = psB.tile([128, qb_size], F32, tag="pav")
                    psum = psC.tile([128, qb_size], F32, tag="sum")
                    prev = None
                    pairs = []
                    for g in range(ngroups):
                        sc = psA.tile([128, gk * 512], F32, tag="sc")
                        for j in range(gk):
                            kt = g * gk + j
                            nc.tensor.matmul(
                                sc[:, j * 512:(j + 1) * 512],
                                kT[:, kt * 128:(kt + 1) * 128],
                                rhs_q,
                            )
                        # drain previous group's exp through AV/sum matmuls so
                        # PE's scores for group g+1 overlap ACT's exp of group g
                        if prev is not None:
                            pe_consume(prev, pav, psum, vsrc, pairs)
                        et = epool.tile([128, gk * 512], F32R, tag="e")
                        nc.scalar.activation(et[:], sc[:], EXP, scale=SCALE)
                        prev = (et, g)
                    pe_consume(prev, pav, psum, vsrc, pairs)
                    rho = small.tile([128, qb_size], F32, tag=f"rho{comp}")
                    nc.vector.reciprocal_approx_fast(rho[:], psum[:])
                    pavs.append(pav)
                    rhos.append(rho)

                t0 = small.tile([128, qb_size], F32, tag="t0")
                nc.vector.tensor_tensor(out=t0[:], in0=pavs[0][:], in1=rhos[0][:], op=MULT)
                t1 = small.tile([128, qb_size], F32, tag="t1")
                nc.vector.tensor_tensor(out=t1[:], in0=pavs[1][:], in1=rhos[1][:], op=MULT)
                o = small.tile([128, qb_size], F32, tag="o")
                nc.vector.tensor_tensor(out=o[:], in0=t0[:], in1=t1[:], op=ADD)

                osb = small.tile([128, njb, 128], F32, tag="osb")
                pt = psA.tile([128, gk * 512], F32, tag="sc")
                for j in range(njb):
                    nc.tensor.transpose(
                        pt[:, j * 128:(j + 1) * 128], o[:, j * 128:(j + 1) * 128],
                        ident[:],
                    )
                    nc.vector.tensor_copy(osb[:, j, :], pt[:, j * 128:(j + 1) * 128])
                nc.sync.dma_start(ov[qb], osb[:])

    nc.compile()
    return nc


def host_consts():
    ident = np.eye(128, dtype=np.float32)
    # lhsT for Qswap^T = M' @ Qneg^T: lhsT[2d+1, 2d] = -1, lhsT[2d, 2d+1] = +1
    swapneg = np.zeros((128, 128), dtype=np.float32)
    idx = np.arange(0, 128, 2)
    swapneg[idx + 1, idx] = -1.0
    swapneg[idx, idx + 1] = 1.0
    onesm = np.ones((128, 128), dtype=np.float32)
    sign = np.empty((128, 1), dtype=np.float32)
    sign[0::2] = 1.0
    sign[1::2] = -1.0
    return {"ident": ident, "swapneg": swapneg, "onesm": onesm, "sign": sign}


_LAST_RESULTS = [None]  # BassKernelResults stash for test harness introspection


def kernel(queries, keys, values):
    from concourse.bass_utils import run_bass_kernel_spmd

    queries = np.ascontiguousarray(np.asarray(queries, dtype=np.float32))
    keys = np.ascontiguousarray(np.asarray(keys, dtype=np.float32))
    values = np.ascontiguousarray(np.asarray(values, dtype=np.float32))
    assert queries.shape == (B, S, D, 2), queries.shape

    nc = build_nc()
    consts = host_consts()
    halves = S // (NCORES // B)  # 2048 rows per core
    in_maps = []
    for c in range(NCORES):
        b, h = c // 2, c % 2
        in_maps.append({
            "q": queries[b, h * halves:(h + 1) * halves].reshape(SQ, W),
            "k": keys[b].reshape(S, W),
            "v": values[b].reshape(S, W),
            **consts,
        })
    res = run_bass_kernel_spmd(
        nc, in_maps, list(range(NCORES)),
        trace=bool(int(os.environ.get("KERNEL_TRACE", "0"))),
    )
    _LAST_RESULTS[0] = res
    out = np.empty((B, S, D, 2), dtype=np.float32)
    for c in range(NCORES):
        b, h = c // 2, c % 2
        out[b, h * halves:(h + 1) * halves] = res.results[c]["out"].reshape(halves, D, 2)
    return out


# revision 7
# speedup vs baseline: 1.2109x; 1.1472x over previous
"""Complex dot-product attention on 8 Trainium2 NeuronCores.

Reference computation (per batch b):
    sr = (qr @ kr^T - qi @ ki^T) / sqrt(D)      si = (qr @ ki^T + qi @ kr^T) / sqrt(D)
    ar = softmax(sr, axis=k)                    ai = softmax(si, axis=k)
    out_r = ar @ vr - ai @ vi                   out_i = ar @ vi + ai @ vr

Shapes: q/k/v [B=4, S=4096, D=64, 2] fp32, interleaved (real, imag) last dim.

Sharding: data-parallel over batch x sequence-parallel over query rows.
Core c handles batch b = c//2, query rows [h*2048, (h+1)*2048) with h = c%2,
and all 4096 keys of that batch (K/V replicated per batch pair). No
collectives; the host slices inputs per core and concatenates outputs.

Per-core kernel math trick: with Q, K, V kept in their NATURAL interleaved
layout ([s, 2d] where col 2d = real_d, col 2d+1 = imag_d):
    sr[q,k] = sum_{2d} Qneg[q,:] * K[k,:]   with Qneg = [qr0, -qi0, qr1, -qi1, ...]
    si[q,k] = sum_{2d} Qswap[q,:] * K[k,:]  with Qswap = [qi0, qr0, qi1, qr1, ...]
so both score components contract over the full 128-wide interleaved axis
against the SAME natural K. Scores are computed TRANSPOSED ([k, q]) so that
the attention matmul (contraction over k) can consume the exp'd scores
directly from SBUF as the moving operand:
    P_a[m, q] = sum_k V[k, m]  * Er[k, q]   (V natural as stationary)
    P_b[m, q] = sum_k V2[k, m] * Ei[k, q]   (V2 = [-vi0, vr0, -vi1, vr1, ...])
    out_T[m, q] = P_a[m,q] / sum_r[q] + P_b[m,q] / sum_i[q]
which lands rows m = (d, complex)-interleaved, exactly the HBM layout after a
final 128x128 PE transpose. Softmax skips max-subtraction (scores are
O(+-6) for randn inputs; exp stays comfortably inside fp32 range).
"""

import os

import numpy as np

import concourse.bass as bass
import concourse.mybir as mybir
import concourse.tile as tile
from concourse import bacc

F32 = mybir.dt.float32
F32R = mybir.dt.float32r
EXP = mybir.ActivationFunctionType.Exp
MULT = mybir.AluOpType.mult
ADD = mybir.AluOpType.add

B, S, D = 4, 4096, 64
W = 2 * D  # 128 interleaved columns
NCORES = 8
SQ = B * S // NCORES  # 2048 query rows per core
SCALE = 1.0 / float(np.sqrt(D))


def build_nc(sq=SQ, sk=S, gk=2, qb_size=512):
    """Build the per-core SPMD bass program."""
    nq = sq // 128   # q 128-row chunks
    nk = sk // 128   # k tiles
    nqb = sq // qb_size
    njb = qb_size // 128
    ngroups = nk // gk

    nc = bacc.Bacc(target_bir_lowering=False)

    q_d = nc.declare_dram_parameter("q", [sq, W], F32, isOutput=False)
    k_d = nc.declare_dram_parameter("k", [sk, W], F32, isOutput=False)
    v_d = nc.declare_dram_parameter("v", [sk, W], F32, isOutput=False)
    ident_d = nc.declare_dram_parameter("ident", [128, 128], F32, isOutput=False)
    swapneg_d = nc.declare_dram_parameter("swapneg", [128, 128], F32R, isOutput=False)
    onesm_d = nc.declare_dram_parameter("onesm", [128, 128], F32R, isOutput=False)
    sign_d = nc.declare_dram_parameter("sign", [128, 1], F32, isOutput=False)
    out_d = nc.declare_dram_parameter("out", [sq, W], F32, isOutput=True)

    qv = q_d.rearrange("(c p) n -> p c n", p=128)  # [128, nq, 128]
    kv = k_d.rearrange("(c p) n -> p c n", p=128)
    vv = v_d.rearrange("(c p) n -> p c n", p=128)
    # out row = a*qb_size + j*128 + p
    ov = out_d.rearrange("(a j p) n -> a p j n", p=128, j=njb)

    with tile.TileContext(nc) as tc:
        with (
            tc.tile_pool(name="const", bufs=1) as constp,
            tc.tile_pool(name="big", bufs=1) as big,
            tc.tile_pool(name="epool", bufs=6) as epool,
            tc.tile_pool(name="small", bufs=2) as small,
            # PSUM budget: 8 banks of [128 x 512 fp32].
            tc.tile_pool(name="psA", bufs=2, space=bass.MemorySpace.PSUM) as psA,  # scores: 2x2 banks
            tc.tile_pool(name="psB", bufs=3, space=bass.MemorySpace.PSUM) as psB,  # AV accum: 3x1
            tc.tile_pool(name="psC", bufs=1, space=bass.MemorySpace.PSUM) as psC,  # sums: 1x1
        ):
            ident = constp.tile([128, 128], F32, tag="ident")
            nc.sync.dma_start(ident[:], ident_d[:])
            swapneg = constp.tile([128, 128], F32R, tag="swapneg")
            nc.sync.dma_start(swapneg[:], swapneg_d[:])
            onesm = constp.tile([128, 128], F32R, tag="onesm")
            nc.sync.dma_start(onesm[:], onesm_d[:])
            sign = constp.tile([128, 1], F32, tag="sign")
            nc.sync.dma_start(sign[:], sign_d[:])

            CH = 4  # tiles per DMA chunk
            kchunks, qchunks, vchunks = [], [], []
            for c0 in range(0, nk, CH):
                t = big.tile([128, min(CH, nk - c0), 128], F32, tag=f"knat{c0}")
                nc.sync.dma_start(t[:], kv[:, c0:c0 + t.shape[1], :])
                kchunks.append(t)
            for c0 in range(0, nq, CH):
                t = big.tile([128, min(CH, nq - c0), 128], F32, tag=f"qnat{c0}")
                nc.sync.dma_start(t[:], qv[:, c0:c0 + t.shape[1], :])
                qchunks.append(t)

            # K^T: [2d, k]
            kT = big.tile([128, sk], F32R, tag="kT")
            for c in range(nk):
                ps = psA.tile([128, 128], F32, tag="sc")
                nc.tensor.transpose(ps[:], kchunks[c // CH][:, c % CH, :], ident[:])
                nc.vector.tensor_copy(kT[:, c * 128:(c + 1) * 128], ps[:])

            # Qneg^T: [2d, q] with odd partitions negated (sign = +1/-1 per partition)
            qnegT = big.tile([128, sq], F32R, tag="qnegT")
            for c in range(nq):
                ps = psA.tile([128, 128], F32, tag="sc")
                nc.tensor.transpose(ps[:], qchunks[c // CH][:, c % CH, :], ident[:])
                nc.vector.tensor_scalar(
                    out=qnegT[:, c * 128:(c + 1) * 128], in0=ps[:],
                    scalar1=sign[:], scalar2=None, op0=MULT,
                )

            vnat = big.tile([128, nk, 128], F32, tag="vnat")
            for c0 in range(0, nk, CH):
                ce = min(c0 + CH, nk)
                nc.sync.dma_start(vnat[:, c0:ce, :], vv[:, c0:ce, :])

            # Qswap^T = M' @ Qneg^T (M' undoes the sign and swaps even/odd partitions)
            qswapT = big.tile([128, sq], F32R, tag="qswapT")
            for n0 in range(0, sq, 512):
                ps = psB.tile([128, 512], F32, tag="pav")
                nc.tensor.matmul(
                    ps[:], swapneg[:],
                    qnegT[:, n0:n0 + 512],
                )
                nc.vector.tensor_copy(qswapT[:, n0:n0 + 512], ps[:])

            # V rounded to fp32r for the AV-matmul stationary
            v1r = big.tile([128, nk, 128], F32R, tag="v1r")
            nc.vector.tensor_copy(v1r[:], vnat[:])

            # V2 = [-vi, vr] interleaved
            v2 = big.tile([128, nk, 128], F32R, tag="v2")
            vp = vnat.rearrange("p c (d two) -> p c d two", two=2)
            v2p = v2.rearrange("p c (d two) -> p c d two", two=2)
            nc.vector.tensor_scalar(
                out=v2p[:, :, :, 0], in0=vp[:, :, :, 1],
                scalar1=-1.0, scalar2=None, op0=MULT,
            )
            nc.vector.tensor_copy(v2p[:, :, :, 1], vp[:, :, :, 0])

            def pe_consume(prev, pav, psum, vsrc, pairs):
                """AV matmuls + pair-reduction for one exp'd score group.

                Denominator: E slices are pair-added (DVE/GpSimd, alternating),
                pairs quad-merged, and only one 128x512 ones-matmul per quad
                streams through the PE -- 4x less PE time than per-kt ones.
                """
                et, g = prev
                for j in range(gk):
                    kt = g * gk + j
                    er = et[:, j * 512:(j + 1) * 512]
                    nc.tensor.matmul(
                        pav[:], vsrc[:, kt, :], er,
                        start=(kt == 0), stop=(kt == nk - 1),
                    )
                pr = small.tile([128, qb_size], F32R, tag=f"pair{g % 4}")
                nc.vector.tensor_tensor(out=pr[:], in0=et[:, 0:512], in1=et[:, 512:1024], op=ADD)
                nc.tensor.matmul(
                    psum[:], onesm[:], pr[:],
                    start=(g == 0), stop=(g == ngroups - 1),
                )

            for qb in range(nqb):
                pavs, rhos = [], []
                for comp in range(2):
                    src = qnegT if comp == 0 else qswapT
                    rhs_q = src[:, qb * qb_size:(qb + 1) * qb_size]
                    vsrc = v1r if comp == 0 else v2
                    pav = psB.tile([128, qb_size], F32, tag="pav")
                    psum = psC.tile([128, qb_size], F32, tag="sum")
                    prev = None
                    pairs = []
                    for g in range(ngroups):
                        sc = psA.tile([128, gk * 512], F32, tag="sc")
                        for j in range(gk):
                            kt = g * gk + j
                            nc.tensor.matmul(
                                sc[:, j * 512:(j + 1) * 512],
                                kT[:, kt * 128:(kt + 1) * 128],
                                rhs_q,
                            )
                        # drain previous group's exp through AV/sum matmuls so
                        # PE's scores for group g+1 overlap ACT's exp of group g
                        if prev is not None:
                            pe_consume(prev, pav, psum, vsrc, pairs)
                        et = epool.tile([128, gk * 512], F32R, tag="e")
                        nc.scalar.activation(et[:], sc[:], EXP, scale=SCALE)
                        prev = (et, g)
                    pe_consume(prev, pav, psum, vsrc, pairs)
                    rho = small.tile([128, qb_size], F32, tag=f"rho{comp}")
                    nc.vector.reciprocal_approx_fast(rho[:], psum[:])
                    pavs.append(pav)
                    rhos.append(rho)

                t0 = small.tile([128, qb_size], F32, tag="t0")
                nc.vector.tensor_tensor(out=t0[:], in0=pavs[0][:], in1=rhos[0][:], op=MULT)
                t1 = small.tile([128, qb_size], F32, tag="t1")
                nc.vector.tensor_tensor(out=t1[:], in0=pavs[1][:], in1=rhos[1][:], op=MULT)
                o = small.tile([128, qb_size], F32, tag="o")
                nc.vector.tensor_tensor(out=o[:], in0=t0[:], in1=t1[:], op=ADD)

                osb = small.tile([128, njb, 128], F32, tag="osb")
                pt = psA.tile([128, gk * 512], F32, tag="sc")
                for j in range(njb):
                    nc.tensor.transpose(
                        pt[:, j * 128:(j + 1) * 128], o[:, j * 128:(j + 1) * 128],
                        ident[:],
                    )
                    nc.vector.tensor_copy(osb[:, j, :], pt[:, j * 128:(j + 1) * 128])
                nc.sync.dma_start(ov[qb], osb[:])

    nc.compile()
    return nc


def host_consts():
    ident = np.eye(128, dtype=np.float32)
    # lhsT for Qswap^T = M' @ Qneg^T: lhsT[2d+1, 2d] = -1, lhsT[2d, 2d+1] = +1
    swapneg = np.zeros((128, 128), dtype=np.float32)
    idx = np.arange(0, 128, 2)
    swapneg[idx + 1, idx] = -1.0
    swapneg[idx, idx + 1] = 1.0
    onesm = np.ones((128, 128), dtype=np.float32)
    sign = np.empty((128, 1), dtype=np.float32)
    sign[0::2] = 1.0
    sign[1::2] = -1.0
    return {"ident": ident, "swapneg": swapneg, "onesm": onesm, "sign": sign}


_LAST_RESULTS = [None]  # BassKernelResults stash for test harness introspection


def kernel(queries, keys, values):
    from concourse.bass_utils import run_bass_kernel_spmd

    queries = np.ascontiguousarray(np.asarray(queries, dtype=np.float32))
    keys = np.ascontiguousarray(np.asarray(keys, dtype=np.float32))
    values = np.ascontiguousarray(np.asarray(values, dtype=np.float32))
    assert queries.shape == (B, S, D, 2), queries.shape

    nc = build_nc()
    consts = host_consts()
    halves = S // (NCORES // B)  # 2048 rows per core
    in_maps = []
    for c in range(NCORES):
        b, h = c // 2, c % 2
        in_maps.append({
            "q": queries[b, h * halves:(h + 1) * halves].reshape(SQ, W),
            "k": keys[b].reshape(S, W),
            "v": values[b].reshape(S, W),
            **consts,
        })
    res = run_bass_kernel_spmd(
        nc, in_maps, list(range(NCORES)),
        trace=bool(int(os.environ.get("KERNEL_TRACE", "0"))),
    )
    _LAST_RESULTS[0] = res
    out = np.empty((B, S, D, 2), dtype=np.float32)
    for c in range(NCORES):
        b, h = c // 2, c % 2
        out[b, h * halves:(h + 1) * halves] = res.results[c]["out"].reshape(halves, D, 2)
    return out


# revision 8
# speedup vs baseline: 1.2844x; 1.0606x over previous
"""Complex dot-product attention on 8 Trainium2 NeuronCores.

Reference computation (per batch b):
    sr = (qr @ kr^T - qi @ ki^T) / sqrt(D)      si = (qr @ ki^T + qi @ kr^T) / sqrt(D)
    ar = softmax(sr, axis=k)                    ai = softmax(si, axis=k)
    out_r = ar @ vr - ai @ vi                   out_i = ar @ vi + ai @ vr

Shapes: q/k/v [B=4, S=4096, D=64, 2] fp32, interleaved (real, imag) last dim.

Sharding: data-parallel over batch x sequence-parallel over query rows.
Core c handles batch b = c//2, query rows [h*2048, (h+1)*2048) with h = c%2,
and all 4096 keys of that batch (K/V replicated per batch pair). No
collectives; the host slices inputs per core and concatenates outputs.

Per-core kernel math trick: with Q, K, V kept in their NATURAL interleaved
layout ([s, 2d] where col 2d = real_d, col 2d+1 = imag_d):
    sr[q,k] = sum_{2d} Qneg[q,:] * K[k,:]   with Qneg = [qr0, -qi0, qr1, -qi1, ...]
    si[q,k] = sum_{2d} Qswap[q,:] * K[k,:]  with Qswap = [qi0, qr0, qi1, qr1, ...]
so both score components contract over the full 128-wide interleaved axis
against the SAME natural K. Scores are computed TRANSPOSED ([k, q]) so that
the attention matmul (contraction over k) can consume the exp'd scores
directly from SBUF as the moving operand:
    P_a[m, q] = sum_k V[k, m]  * Er[k, q]   (V natural as stationary)
    P_b[m, q] = sum_k V2[k, m] * Ei[k, q]   (V2 = [-vi0, vr0, -vi1, vr1, ...])
    out_T[m, q] = P_a[m,q] / sum_r[q] + P_b[m,q] / sum_i[q]
which lands rows m = (d, complex)-interleaved, exactly the HBM layout after a
final 128x128 PE transpose. Softmax skips max-subtraction (scores are
O(+-6) for randn inputs; exp stays comfortably inside fp32 range).
"""

import os

import numpy as np

import concourse.bass as bass
import concourse.mybir as mybir
import concourse.tile as tile
from concourse import bacc

F32 = mybir.dt.float32
F32R = mybir.dt.float32r
EXP = mybir.ActivationFunctionType.Exp
MULT = mybir.AluOpType.mult
ADD = mybir.AluOpType.add

B, S, D = 4, 4096, 64
W = 2 * D  # 128 interleaved columns
NCORES = 8
SQ = B * S // NCORES  # 2048 query rows per core
SCALE = 1.0 / float(np.sqrt(D))


def build_nc(sq=SQ, sk=S, gk=2, qb_size=512):
    """Build the per-core SPMD bass program."""
    nq = sq // 128   # q 128-row chunks
    nk = sk // 128   # k tiles
    nqb = sq // qb_size
    njb = qb_size // 128
    ngroups = nk // gk
    assert ngroups % 2 == 0

    nc = bacc.Bacc(target_bir_lowering=False)

    q_d = nc.declare_dram_parameter("q", [sq, W], F32, isOutput=False)
    k_d = nc.declare_dram_parameter("k", [sk, W], F32, isOutput=False)
    v_d = nc.declare_dram_parameter("v", [sk, W], F32, isOutput=False)
    ident_d = nc.declare_dram_parameter("ident", [128, 128], F32, isOutput=False)
    swapneg_d = nc.declare_dram_parameter("swapneg", [128, 128], F32R, isOutput=False)
    onesm_d = nc.declare_dram_parameter("onesm", [128, 128], F32R, isOutput=False)
    sign_d = nc.declare_dram_parameter("sign", [128, 1], F32, isOutput=False)
    out_d = nc.declare_dram_parameter("out", [sq, W], F32, isOutput=True)

    qv = q_d.rearrange("(c p) n -> p c n", p=128)  # [128, nq, 128]
    kv = k_d.rearrange("(c p) n -> p c n", p=128)
    vv = v_d.rearrange("(c p) n -> p c n", p=128)
    # out row = a*qb_size + j*128 + p
    ov = out_d.rearrange("(a j p) n -> a p j n", p=128, j=njb)

    with tile.TileContext(nc) as tc:
        with (
            tc.tile_pool(name="const", bufs=1) as constp,
            tc.tile_pool(name="big", bufs=1) as big,
            tc.tile_pool(name="epool", bufs=6) as epool,
            tc.tile_pool(name="small", bufs=2) as small,
            # PSUM budget: 8 banks of [128 x 512 fp32].
            tc.tile_pool(name="psA", bufs=2, space=bass.MemorySpace.PSUM) as psA,  # scores: 2x2 banks
            tc.tile_pool(name="psB", bufs=3, space=bass.MemorySpace.PSUM) as psB,  # AV accum + out-tr: 3x1
            tc.tile_pool(name="psC", bufs=1, space=bass.MemorySpace.PSUM) as psC,  # sums: 1x1
        ):
            CH = 4  # tiles per DMA chunk
            # sync queue: ident first (transposes need it), then K chunks.
            # scalar (2nd HWDGE queue): Q chunks + remaining consts, concurrently.
            ident = constp.tile([128, 128], F32, tag="ident")
            nc.sync.dma_start(ident[:], ident_d[:])
            kchunks, qchunks = [], []
            for c0 in range(0, nk, CH):
                t = big.tile([128, min(CH, nk - c0), 128], F32, tag=f"knat{c0}")
                nc.sync.dma_start(t[:], kv[:, c0:c0 + t.shape[1], :])
                kchunks.append(t)
            for c0 in range(0, nq, CH):
                t = big.tile([128, min(CH, nq - c0), 128], F32, tag=f"qnat{c0}")
                nc.scalar.dma_start(t[:], qv[:, c0:c0 + t.shape[1], :])
                qchunks.append(t)
            sign = constp.tile([128, 1], F32, tag="sign")
            nc.scalar.dma_start(sign[:], sign_d[:])
            swapneg = constp.tile([128, 128], F32R, tag="swapneg")
            nc.scalar.dma_start(swapneg[:], swapneg_d[:])
            onesm = constp.tile([128, 128], F32R, tag="onesm")
            nc.scalar.dma_start(onesm[:], onesm_d[:])

            # K^T: [2d, k]
            kT = big.tile([128, sk], F32R, tag="kT")
            for c in range(nk):
                ps = psA.tile([128, 128], F32, tag="sc")
                nc.tensor.transpose(ps[:], kchunks[c // CH][:, c % CH, :], ident[:])
                nc.vector.tensor_copy(kT[:, c * 128:(c + 1) * 128], ps[:])

            # Qneg^T: [2d, q] with odd partitions negated (sign = +1/-1 per partition)
            qnegT = big.tile([128, sq], F32R, tag="qnegT")
            for c in range(nq):
                ps = psA.tile([128, 128], F32, tag="sc")
                nc.tensor.transpose(ps[:], qchunks[c // CH][:, c % CH, :], ident[:])
                nc.vector.tensor_scalar(
                    out=qnegT[:, c * 128:(c + 1) * 128], in0=ps[:],
                    scalar1=sign[:], scalar2=None, op0=MULT,
                )

            # Qswap^T = M' @ Qneg^T (M' undoes the sign and swaps even/odd partitions)
            qswapT = big.tile([128, sq], F32R, tag="qswapT")
            for n0 in range(0, sq, 512):
                ps = psB.tile([128, 512], F32, tag="pav")
                nc.tensor.matmul(
                    ps[:], swapneg[:],
                    qnegT[:, n0:n0 + 512],
                )
                nc.vector.tensor_copy(qswapT[:, n0:n0 + 512], ps[:])

            # V loads + per-chunk prep: V1 = fp32r-rounded copy (AV stationary),
            # V2 = [-vi, vr] interleaved. Chunked so DVE work stays fine-grained.
            vnat = big.tile([128, nk, 128], F32, tag="vnat")
            v1r = big.tile([128, nk, 128], F32R, tag="v1r")
            v2 = big.tile([128, nk, 128], F32R, tag="v2")
            vp = vnat.rearrange("p c (d two) -> p c d two", two=2)
            v2p = v2.rearrange("p c (d two) -> p c d two", two=2)
            for c0 in range(0, nk, CH):
                ce = min(c0 + CH, nk)
                nc.sync.dma_start(vnat[:, c0:ce, :], vv[:, c0:ce, :])
                nc.vector.tensor_copy(v1r[:, c0:ce, :], vnat[:, c0:ce, :])
                nc.vector.tensor_scalar(
                    out=v2p[:, c0:ce, :, 0], in0=vp[:, c0:ce, :, 1],
                    scalar1=-1.0, scalar2=None, op0=MULT,
                )
                nc.vector.tensor_copy(v2p[:, c0:ce, :, 1], vp[:, c0:ce, :, 0])

            def pe_consume(prev, pav, psum, vsrc, pairs):
                """AV matmuls + denominator reduction for one exp'd group.

                Denominator: E slices pair-added then quad-merged on DVE; one
                128x512 ones-matmul per quad (4 k-tiles) streams through PE.
                """
                et, g = prev
                for j in range(gk):
                    kt = g * gk + j
                    er = et[:, j * 512:(j + 1) * 512]
                    nc.tensor.matmul(
                        pav[:], vsrc[:, kt, :], er,
                        start=(kt == 0), stop=(kt == nk - 1),
                    )
                pr = small.tile([128, qb_size], F32R, tag=f"pair{g % 4}")
                nc.vector.tensor_tensor(out=pr[:], in0=et[:, 0:512], in1=et[:, 512:1024], op=ADD)
                pairs.append(pr)
                if len(pairs) == 2:
                    qd = small.tile([128, qb_size], F32R, tag=f"quad{(g // 2) % 3}")
                    nc.vector.tensor_tensor(out=qd[:], in0=pairs[0][:], in1=pairs[1][:], op=ADD)
                    pairs.clear()
                    h = g // 2
                    nc.tensor.matmul(
                        psum[:], onesm[:], qd[:],
                        start=(h == 0), stop=(h == ngroups // 2 - 1),
                    )

            def make_qb_tail(qb, pavs, rhos):
                def run():
                    t0 = small.tile([128, qb_size], F32, tag="t0")
                    nc.vector.tensor_tensor(out=t0[:], in0=pavs[0][:], in1=rhos[0][:], op=MULT)
                    t1 = small.tile([128, qb_size], F32, tag="t1")
                    nc.vector.tensor_tensor(out=t1[:], in0=pavs[1][:], in1=rhos[1][:], op=MULT)
                    o = small.tile([128, qb_size], F32, tag="o")
                    nc.vector.tensor_tensor(out=o[:], in0=t0[:], in1=t1[:], op=ADD)

                    osb = small.tile([128, njb, 128], F32, tag="osb")
                    pt = psB.tile([128, 512], F32, tag="pav")
                    for j in range(njb):
                        nc.tensor.transpose(
                            pt[:, j * 128:(j + 1) * 128], o[:, j * 128:(j + 1) * 128],
                            ident[:],
                        )
                        nc.vector.tensor_copy(osb[:, j, :], pt[:, j * 128:(j + 1) * 128])
                    nc.sync.dma_start(ov[qb], osb[:])
                return run

            pending = None
            defer_g = min(2, ngroups - 1)
            for qb in range(nqb):
                pavs, rhos = [], []
                for comp in range(2):
                    src = qnegT if comp == 0 else qswapT
                    rhs_q = src[:, qb * qb_size:(qb + 1) * qb_size]
                    vsrc = v1r if comp == 0 else v2
                    pav = psB.tile([128, qb_size], F32, tag="pav")
                    psum = psC.tile([128, qb_size], F32, tag="sum")
                    prev = None
                    pairs = []
                    for g in range(ngroups):
                        sc = psA.tile([128, gk * 512], F32, tag="sc")
                        for j in range(gk):
                            kt = g * gk + j
                            nc.tensor.matmul(
                                sc[:, j * 512:(j + 1) * 512],
                                kT[:, kt * 128:(kt + 1) * 128],
                                rhs_q,
                            )
                        # drain previous group's exp through AV/sum matmuls so
                        # PE's scores for group g+1 overlap ACT's exp of group g
                        if prev is not None:
                            pe_consume(prev, pav, psum, vsrc, pairs)
                        # previous q-block's combine/store runs here, hidden
                        # behind this block's early matmul stream
                        if pending is not None and comp == 0 and g == defer_g:
                            pending()
                            pending = None
                        et = epool.tile([128, gk * 512], F32R, tag="e")
                        nc.scalar.activation(et[:], sc[:], EXP, scale=SCALE)
                        prev = (et, g)
                    pe_consume(prev, pav, psum, vsrc, pairs)
                    rho = small.tile([128, qb_size], F32, tag=f"rho{comp}")
                    nc.vector.reciprocal_approx_fast(rho[:], psum[:])
                    pavs.append(pav)
                    rhos.append(rho)
                pending = make_qb_tail(qb, pavs, rhos)
            pending()

    nc.compile()
    return nc


def host_consts():
    ident = np.eye(128, dtype=np.float32)
    # lhsT for Qswap^T = M' @ Qneg^T: lhsT[2d+1, 2d] = -1, lhsT[2d, 2d+1] = +1
    swapneg = np.zeros((128, 128), dtype=np.float32)
    idx = np.arange(0, 128, 2)
    swapneg[idx + 1, idx] = -1.0
    swapneg[idx, idx + 1] = 1.0
    onesm = np.ones((128, 128), dtype=np.float32)
    sign = np.empty((128, 1), dtype=np.float32)
    sign[0::2] = 1.0
    sign[1::2] = -1.0
    return {"ident": ident, "swapneg": swapneg, "onesm": onesm, "sign": sign}


_LAST_RESULTS = [None]  # BassKernelResults stash for test harness introspection


def kernel(queries, keys, values):
    from concourse.bass_utils import run_bass_kernel_spmd

    queries = np.ascontiguousarray(np.asarray(queries, dtype=np.float32))
    keys = np.ascontiguousarray(np.asarray(keys, dtype=np.float32))
    values = np.ascontiguousarray(np.asarray(values, dtype=np.float32))
    assert queries.shape == (B, S, D, 2), queries.shape

    nc = build_nc()
    consts = host_consts()
    halves = S // (NCORES // B)  # 2048 rows per core
    in_maps = []
    for c in range(NCORES):
        b, h = c // 2, c % 2
        in_maps.append({
            "q": queries[b, h * halves:(h + 1) * halves].reshape(SQ, W),
            "k": keys[b].reshape(S, W),
            "v": values[b].reshape(S, W),
            **consts,
        })
    res = run_bass_kernel_spmd(
        nc, in_maps, list(range(NCORES)),
        trace=bool(int(os.environ.get("KERNEL_TRACE", "0"))),
    )
    _LAST_RESULTS[0] = res
    out = np.empty((B, S, D, 2), dtype=np.float32)
    for c in range(NCORES):
        b, h = c // 2, c % 2
        out[b, h * halves:(h + 1) * halves] = res.results[c]["out"].reshape(halves, D, 2)
    return out
